# revision 13
# baseline (speedup 1.0000x reference)
"""Trainium2 Bass kernel for nn_Attention2 (gnn message passing, N=1M, K=9, C=20).

Strategy: data-parallel over points (8 cores, 125k points each). KNN indices are
uniform-random global gathers; each core gathers from full replicated tables via
indirect DMA. Gather indices are batched GB=108 per indirect-DMA instruction
(the SWDGE fixed cost is ~1us per instruction, so per-row gathers are issue
bound; batched gathers are DMA-descriptor bound). The gathered pass-1 data is
dumped to DRAM in fp16 so the attention pass re-reads it sequentially instead
of re-gathering. Batch-norm statistics are computed as per-core partial sums
and combined on host (f64) between launches; all heavy math runs on device.

The attention math is refactored so the BN1 affine is applied via per-point
small tensors instead of full [N,K,C] ops:
  y[n,k,c]  = G[idx[n,k],c] - P1[n,c]          (G, P1 from L1 matmuls)
  xhat      = a1*y + d1 = a1 * u,  u = r - q', q' = P1 - d1/a1
  w[n,k]    = sum_c xhat[n,k,c]*xhat[n,0,c] = sum_c u[n,k,c] * (a1^2 u[n,0,c])
  x2[n,c]   = a1_c * sum_k u[n,k,c] w[n,k]
which needs only 3 full-size multiplies (DVE) + 2 full-size reduces (GpSimd).
"""

import sys

sys.path.insert(0, "/opt/trn_rl_repo")

import numpy as np

import concourse.bass as bass
import concourse.bacc as bacc
import concourse.tile as tile
from concourse import mybir
from concourse.bass_utils import run_bass_kernel_spmd

F32 = mybir.dt.float32
F16 = mybir.dt.float16
I32 = mybir.dt.int32
AXX = mybir.AxisListType.X
MUL = mybir.AluOpType.mult
ADD = mybir.AluOpType.add
SUB = mybir.AluOpType.subtract

KNN = 9
C = 20          # channels (in_channel == inner_channel == 20)
CIN3 = C + 3    # conv1 input channels
NCORES = 8
EPS = 1e-5
MMB = 5                    # row-blocks per L1 matmul (5 x 23 = 115 partitions)
L1ROWS = 128 * MMB         # rows per L1 iteration (640)
QCH = 512                  # column chunk for L5/L6
CP = 48                    # points per partition per gather tile
PPT = 128 * CP             # points per gather tile
GB = 108                   # gather indices per indirect-DMA instruction
_prog_cache = {}


def _set_n(n):
    """Set problem size and derived constants (test hook; default N=1M)."""
    global N, NPC, NT, NPAD, SLOTS, FR, NL1, NPAD1
    N = n
    NPC = N // NCORES          # points per core
    NT = -(-NPC // PPT)        # gather tiles per core
    NPAD = NT * PPT            # padded points per core (gather layout)
    SLOTS = CP * KNN           # gather slots per partition per tile (432)
    FR = SLOTS * C             # gathered floats per partition per tile (8640)
    NL1 = -(-NPC // L1ROWS)    # L1 iterations
    NPAD1 = NL1 * L1ROWS
    _prog_cache.clear()



PROFILE = False            # test.py sets this; grader path keeps False
_last_exec_ns = {}


_set_n(1_000_000)


def _v(t_ap, dims, extra_offset=0):
    """View a tile/tensor AP with explicit free-dim (step, count) pairs."""
    return bass.AP(
        tensor=t_ap.tensor,
        offset=t_ap.offset + extra_offset,
        ap=[t_ap.ap[0]] + [[s, c] for s, c in dims],
    )


def _dram_v(t, offset, dims):
    """Arbitrary affine view of a DRAM tensor (element offset, [step,count] dims)."""
    return bass.AP(tensor=t, offset=offset, ap=[[s, c] for s, c in dims])


def _gather(nc, r, table, it):
    """Gather SLOTS rows of C elements per partition from table. The HW
    consumes exactly one index per partition per indirect-DMA instruction
    (measured: multi-index offset APs silently use only idx[p,0]), so this
    is SLOTS instructions; each costs ~1.4us of Pool-engine descriptor
    generation, which is the kernel's overall bottleneck."""
    for s in range(SLOTS):
        nc.gpsimd.indirect_dma_start(
            out=r[:, s * C:(s + 1) * C],
            out_offset=None,
            in_=table,
            in_offset=bass.IndirectOffsetOnAxis(ap=it[:, s:s + 1], axis=0),
        )


# --------------------------------------------------------------------------- L1
def _build_l1():
    nc = bacc.Bacc("TRN2", target_bir_lowering=False, debug=False, num_devices=1)
    xt = nc.dram_tensor("xt", [CIN3, NPAD1], F32, kind="ExternalInput")
    wbd_g = nc.dram_tensor("wbd_g", [CIN3 * MMB, C * MMB], F32, kind="ExternalInput").ap()
    wbd_p = nc.dram_tensor("wbd_p", [CIN3 * MMB, C * MMB], F32, kind="ExternalInput").ap()
    g_out = nc.dram_tensor("g_out", [NPAD1, C], F32, kind="ExternalOutput")
    p_out = nc.dram_tensor("p_out", [NPAD1, C], F32, kind="ExternalOutput")

    KP = CIN3 * MMB  # 115
    with tile.TileContext(nc) as tc:
        with (
            tc.tile_pool(name="w", bufs=1) as wp,
            tc.tile_pool(name="x", bufs=3) as xp,
            tc.tile_pool(name="o", bufs=3) as op,
            tc.tile_pool(name="ps", bufs=4, space="PSUM") as pp,
        ):
            wg = wp.tile([KP, C * MMB], F32)
            nc.sync.dma_start(out=wg[:], in_=wbd_g)
            wpt = wp.tile([KP, C * MMB], F32)
            nc.sync.dma_start(out=wpt[:], in_=wbd_p)
            for i in range(NL1):
                r0 = i * L1ROWS
                lhs = xp.tile([KP, 128], F32)
                for b in range(MMB):
                    nc.sync.dma_start(
                        out=lhs[CIN3 * b:CIN3 * (b + 1), :],
                        in_=_dram_v(xt, r0 + 128 * b, [[NPAD1, CIN3], [1, 128]]),
                    )
                psg = pp.tile([128, C * MMB], F32, tag="psg")
                nc.tensor.matmul(psg[:], lhsT=lhs[:], rhs=wg[:], start=True, stop=True)
                psp = pp.tile([128, C * MMB], F32, tag="psp")
                nc.tensor.matmul(psp[:], lhsT=lhs[:], rhs=wpt[:], start=True, stop=True)
                og = op.tile([128, C * MMB], F32, tag="og")
                nc.scalar.copy(out=og[:], in_=psg[:])
                opt = op.tile([128, C * MMB], F32, tag="opt")
                nc.scalar.copy(out=opt[:], in_=psp[:])
                dst = [[C, 128], [128 * C, MMB], [1, C]]
                nc.sync.dma_start(out=_dram_v(g_out, r0 * C, dst), in_=og[:])
                nc.sync.dma_start(out=_dram_v(p_out, r0 * C, dst), in_=opt[:])
    nc.compile()
    return nc


# --------------------------------------------------------------------------- L2
def _build_l2():
    nc = bacc.Bacc("TRN2", target_bir_lowering=False, debug=False, num_devices=1)
    table = nc.dram_tensor("table", [N + 1, C], F32, kind="ExternalInput").ap()
    idx = nc.dram_tensor("idx", [NT, 128, SLOTS], I32, kind="ExternalInput").ap()
    p1t = nc.dram_tensor("p1t", [NT, 128, CP * C], F32, kind="ExternalInput").ap()
    dump = nc.dram_tensor("dump", [NT, 128, FR], F16, kind="ExternalOutput").ap()
    stats = nc.dram_tensor("stats", [128, 5 * C], F32, kind="ExternalOutput").ap()

    with tile.TileContext(nc) as tc:
        with (
            tc.tile_pool(name="g", bufs=2) as gp,
            tc.tile_pool(name="h", bufs=2) as hp,
            tc.tile_pool(name="i", bufs=2) as ip,
            tc.tile_pool(name="p1", bufs=2) as p1p,
            tc.tile_pool(name="sc", bufs=2) as scp,
            tc.tile_pool(name="acc", bufs=1) as accp,
        ):
            acc = accp.tile([128, 5 * C], F32)
            nc.vector.memset(acc[:], 0.0)
            for t in range(NT):
                it = ip.tile([128, SLOTS], I32)
                nc.sync.dma_start(out=it[:], in_=idx[t])
                p1 = p1p.tile([128, CP * C], F32)
                nc.sync.dma_start(out=p1[:], in_=p1t[t])
                r = gp.tile([128, FR], F32)
                _gather(nc, r, table, it)
                # fp16 copy of the gathered rows for the dump (ACT engine)
                r16 = hp.tile([128, FR], F16)
                nc.scalar.copy(out=r16[:], in_=r[:])
                # S[j,c] = sum_k r[j,k,c]
                sv = scp.tile([128, CP * C], F32, tag="sv")
                nc.vector.tensor_reduce(
                    out=sv[:], in_=_v(r[:], [[KNN * C, CP], [1, C], [C, KNN]]),
                    axis=AXX, op=ADD)
                # accA += sum_j S
                rA = scp.tile([128, C], F32, tag="rA")
                nc.vector.tensor_reduce(
                    out=rA[:], in_=_v(sv[:], [[1, C], [C, CP]]), axis=AXX, op=ADD)
                nc.vector.tensor_add(out=acc[:, 0:C], in0=acc[:, 0:C], in1=rA[:])
                # accB += sum_j P1*S
                ps = scp.tile([128, CP * C], F32, tag="ps")
                nc.vector.tensor_tensor(out=ps[:], in0=p1[:], in1=sv[:], op=MUL)
                rB = scp.tile([128, C], F32, tag="rB")
                nc.vector.tensor_reduce(
                    out=rB[:], in_=_v(ps[:], [[1, C], [C, CP]]), axis=AXX, op=ADD)
                nc.vector.tensor_add(out=acc[:, C:2 * C], in0=acc[:, C:2 * C], in1=rB[:])
                # accQ += sum_{j,k} r^2
                sq = scp.tile([128, FR], F32, tag="sq")
                nc.scalar.square(out=sq[:], in_=r[:])
                rQ = scp.tile([128, C], F32, tag="rQ")
                nc.vector.tensor_reduce(
                    out=rQ[:], in_=_v(sq[:], [[1, C], [C, SLOTS]]), axis=AXX, op=ADD)
                nc.vector.tensor_add(out=acc[:, 2 * C:3 * C], in0=acc[:, 2 * C:3 * C], in1=rQ[:])
                # accP += sum_j P1 ; accPQ += sum_j P1^2
                rP = scp.tile([128, C], F32, tag="rP")
                nc.vector.tensor_reduce(
                    out=rP[:], in_=_v(p1[:], [[1, C], [C, CP]]), axis=AXX, op=ADD)
                nc.vector.tensor_add(out=acc[:, 3 * C:4 * C], in0=acc[:, 3 * C:4 * C], in1=rP[:])
                p1s = scp.tile([128, CP * C], F32, tag="p1s")
                nc.scalar.square(out=p1s[:], in_=p1[:])
                rPQ = scp.tile([128, C], F32, tag="rPQ")
                nc.vector.tensor_reduce(
                    out=rPQ[:], in_=_v(p1s[:], [[1, C], [C, CP]]), axis=AXX, op=ADD)
                nc.vector.tensor_add(out=acc[:, 4 * C:5 * C], in0=acc[:, 4 * C:5 * C], in1=rPQ[:])
                nc.sync.dma_start(out=dump[t], in_=r16[:])
            nc.sync.dma_start(out=stats, in_=acc[:])
    nc.compile()
    return nc


# --------------------------------------------------------------------------- L3
def _build_l3():
    nc = bacc.Bacc("TRN2", target_bir_lowering=False, debug=False, num_devices=1)
    dump = nc.dram_tensor("dump", [NT, 128, FR], F16, kind="ExternalInput").ap()
    p1t = nc.dram_tensor("p1t", [NT, 128, CP * C], F32, kind="ExternalInput").ap()
    a1r = nc.dram_tensor("a1r", [128, C], F32, kind="ExternalInput").ap()
    asqr = nc.dram_tensor("asqr", [128, C], F32, kind="ExternalInput").ap()
    dbr = nc.dram_tensor("dbr", [128, C], F32, kind="ExternalInput").ap()
    w_out = nc.dram_tensor("w_out", [NT, 128, SLOTS], F16, kind="ExternalOutput").ap()
    x2_out = nc.dram_tensor("x2_out", [NT, 128, CP * C], F16, kind="ExternalOutput").ap()

    with tile.TileContext(nc) as tc:
        with (
            tc.tile_pool(name="g", bufs=2) as gp,
            tc.tile_pool(name="big", bufs=2) as bigp,
            tc.tile_pool(name="p1", bufs=2) as p1p,
            tc.tile_pool(name="sc", bufs=2) as scp,
            tc.tile_pool(name="cst", bufs=1) as cst,
        ):
            a1 = cst.tile([128, C], F32)
            nc.sync.dma_start(out=a1[:], in_=a1r)
            asq = cst.tile([128, C], F32)
            nc.sync.dma_start(out=asq[:], in_=asqr)
            db = cst.tile([128, C], F32)
            nc.sync.dma_start(out=db[:], in_=dbr)

            # Two-stage software pipeline so the Pool w-reduce of tile t
            # overlaps the DVE tail (pw2/s/x2) of tile t-1.
            st = {}

            def stage_a(t):
                r16 = gp.tile([128, FR], F16, tag="r16")
                nc.sync.dma_start(out=r16[:], in_=dump[t])
                p1 = p1p.tile([128, CP * C], F32)
                nc.sync.dma_start(out=p1[:], in_=p1t[t])
                # q'[j,c] = P1 - d1/a1   (per point j, channel c)
                qp = scp.tile([128, CP * C], F16, tag="qp")
                nc.vector.tensor_tensor(
                    out=qp[:], in0=p1[:],
                    in1=_v(db[:], [[0, CP], [1, C]]), op=SUB)
                # u = r - q'
                u = bigp.tile([128, FR], F16, tag="u")
                nc.vector.tensor_tensor(
                    out=u[:], in0=r16[:],
                    in1=_v(qp[:], [[C, CP], [0, KNN], [1, C]]), op=SUB)
                # z[j,c] = a1^2 * u[j,0,c]
                z = scp.tile([128, CP * C], F16, tag="z")
                nc.vector.tensor_tensor(
                    out=z[:], in0=_v(u[:], [[KNN * C, CP], [1, C]]),
                    in1=_v(asq[:], [[0, CP], [1, C]]), op=MUL)
                # w[j,k] = sum_c u[j,k,c]*z[j,c]
                pw = bigp.tile([128, FR], F16, tag="pw")
                nc.vector.tensor_tensor(
                    out=pw[:], in0=u[:],
                    in1=_v(z[:], [[C, CP], [0, KNN], [1, C]]), op=MUL)
                w = scp.tile([128, SLOTS], F16, tag="w")
                with nc.allow_low_precision(reason="20-term fp16 dot, 2e-2 tol"):
                    nc.vector.tensor_reduce(
                        out=w[:], in_=_v(pw[:], [[KNN * C, CP], [C, KNN], [1, C]]),
                        axis=AXX, op=ADD)
                st[t] = (u, w)

            def stage_b(t):
                u, w = st.pop(t)
                # s[j,c] = sum_k u[j,k,c]*w[j,k] ; x2 = a1*s
                pw2 = bigp.tile([128, FR], F16, tag="pw2")
                nc.vector.tensor_tensor(
                    out=pw2[:], in0=u[:],
                    in1=_v(w[:], [[KNN, CP], [1, KNN], [0, C]]), op=MUL)
                s = scp.tile([128, CP * C], F16, tag="s")
                with nc.allow_low_precision(reason="9-term fp16 sum, 2e-2 tol"):
                    nc.vector.tensor_reduce(
                        out=s[:], in_=_v(pw2[:], [[KNN * C, CP], [1, C], [C, KNN]]),
                        axis=AXX, op=ADD)
                x2 = scp.tile([128, CP * C], F16, tag="x2")
                nc.vector.tensor_tensor(
                    out=x2[:], in0=s[:],
                    in1=_v(a1[:], [[0, CP], [1, C]]), op=MUL)
                nc.sync.dma_start(out=w_out[t], in_=w[:])
                nc.sync.dma_start(out=x2_out[t], in_=x2[:])

            stage_a(0)
            for t in range(1, NT):
                stage_a(t)
                stage_b(t - 1)
            stage_b(NT - 1)
    nc.compile()
    return nc


# --------------------------------------------------------------------------- L4
def _build_l4():
    nc = bacc.Bacc("TRN2", target_bir_lowering=False, debug=False, num_devices=1)
    table = nc.dram_tensor("table", [N + 1, C], F16, kind="ExternalInput").ap()
    idx = nc.dram_tensor("idx", [NT, 128, SLOTS], I32, kind="ExternalInput").ap()
    w_in = nc.dram_tensor("w_in", [NT, 128, SLOTS], F16, kind="ExternalInput").ap()
    x3_out = nc.dram_tensor("x3_out", [NT, 128, CP * C], F32, kind="ExternalOutput").ap()
    stats = nc.dram_tensor("stats", [128, 2 * C], F32, kind="ExternalOutput").ap()

    with tile.TileContext(nc) as tc:
        with (
            tc.tile_pool(name="g", bufs=2) as gp,
            tc.tile_pool(name="i", bufs=2) as ip,
            tc.tile_pool(name="w", bufs=2) as wp,
            tc.tile_pool(name="px", bufs=2) as pxp,
            tc.tile_pool(name="sc", bufs=2) as scp,
            tc.tile_pool(name="acc", bufs=1) as accp,
        ):
            acc = accp.tile([128, 2 * C], F32)
            nc.vector.memset(acc[:], 0.0)
            for t in range(NT):
                it = ip.tile([128, SLOTS], I32)
                nc.sync.dma_start(out=it[:], in_=idx[t])
                wt = wp.tile([128, SLOTS], F16)
                nc.sync.dma_start(out=wt[:], in_=w_in[t])
                r = gp.tile([128, FR], F16)
                _gather(nc, r, table, it)
                # px = r * w  (broadcast over c; f32 — the product tail can
                # exceed fp16 range)
                px = pxp.tile([128, FR], F32, tag="px")
                nc.vector.tensor_tensor(
                    out=px[:], in0=r[:],
                    in1=_v(wt[:], [[KNN, CP], [1, KNN], [0, C]]), op=MUL)
                x3 = scp.tile([128, CP * C], F32, tag="x3")
                nc.vector.tensor_reduce(
                    out=x3[:], in_=_v(px[:], [[KNN * C, CP], [1, C], [C, KNN]]),
                    axis=AXX, op=ADD)
                rA = scp.tile([128, C], F32, tag="rA")
                nc.vector.tensor_reduce(
                    out=rA[:], in_=_v(x3[:], [[1, C], [C, CP]]), axis=AXX, op=ADD)
                nc.vector.tensor_add(out=acc[:, 0:C], in0=acc[:, 0:C], in1=rA[:])
                sq = scp.tile([128, CP * C], F32, tag="sq")
                nc.scalar.square(out=sq[:], in_=x3[:])
                rB = scp.tile([128, C], F32, tag="rB")
                nc.vector.tensor_reduce(
                    out=rB[:], in_=_v(sq[:], [[1, C], [C, CP]]), axis=AXX, op=ADD)
                nc.vector.tensor_add(out=acc[:, C:2 * C], in0=acc[:, C:2 * C], in1=rB[:])
                nc.sync.dma_start(out=x3_out[t], in_=x3[:])
            nc.sync.dma_start(out=stats, in_=acc[:])
    nc.compile()
    return nc


# --------------------------------------------------------------------------- L5
def _build_l5():
    nc = bacc.Bacc("TRN2", target_bir_lowering=False, debug=False, num_devices=1)
    x3t = nc.dram_tensor("x3t", [C, NPC], F32, kind="ExternalInput")
    ft = nc.dram_tensor("ft", [C, NPC], F32, kind="ExternalInput")
    wr1t = nc.dram_tensor("wr1t", [2 * C, C], F32, kind="ExternalInput").ap()
    a2 = nc.dram_tensor("a2", [C, 1], F32, kind="ExternalInput").ap()
    d2 = nc.dram_tensor("d2", [C, 1], F32, kind="ExternalInput").ap()
    br1 = nc.dram_tensor("br1", [C, 1], F32, kind="ExternalInput").ap()
    tt_out = nc.dram_tensor("tt_out", [C, NPC], F32, kind="ExternalOutput")
    stats = nc.dram_tensor("stats", [C, 2], F32, kind="ExternalOutput").ap()

    nq = -(-NPC // QCH)
    with tile.TileContext(nc) as tc:
        with (
            tc.tile_pool(name="u", bufs=3) as up,
            tc.tile_pool(name="o", bufs=3) as op,
            tc.tile_pool(name="sc", bufs=3) as scp,
            tc.tile_pool(name="cst", bufs=1) as cst,
            tc.tile_pool(name="acc", bufs=1) as accp,
            tc.tile_pool(name="ps", bufs=4, space="PSUM") as pp,
        ):
            w1 = cst.tile([2 * C, C], F32)
            nc.sync.dma_start(out=w1[:], in_=wr1t)
            ca2 = cst.tile([C, 1], F32)
            nc.sync.dma_start(out=ca2[:], in_=a2)
            cd2 = cst.tile([C, 1], F32)
            nc.sync.dma_start(out=cd2[:], in_=d2)
            cbr = cst.tile([C, 1], F32)
            nc.sync.dma_start(out=cbr[:], in_=br1)
            acc = accp.tile([C, 2], F32)
            nc.vector.memset(acc[:], 0.0)
            for i in range(nq):
                c0 = i * QCH
                qn = min(QCH, NPC - c0)
                u = up.tile([2 * C, QCH], F32)
                nc.sync.dma_start(
                    out=u[0:C, :qn], in_=_dram_v(x3t, c0, [[NPC, C], [1, qn]]))
                nc.sync.dma_start(
                    out=u[C:2 * C, :qn], in_=_dram_v(ft, c0, [[NPC, C], [1, qn]]))
                nc.scalar.activation(
                    out=u[0:C, :qn], in_=u[0:C, :qn],
                    func=mybir.ActivationFunctionType.Relu,
                    bias=cd2[:], scale=ca2[:])
                ps = pp.tile([C, QCH], F32)
                nc.tensor.matmul(ps[:, :qn], lhsT=w1[:], rhs=u[:, :qn],
                                 start=True, stop=True)
                tt = op.tile([C, QCH], F32)
                nc.vector.tensor_scalar_add(out=tt[:, :qn], in0=ps[:, :qn], scalar1=cbr[:])
                rs = scp.tile([C, 1], F32, tag="rs")
                nc.vector.tensor_reduce(out=rs[:], in_=tt[:, :qn], axis=AXX, op=ADD)
                nc.vector.tensor_add(out=acc[:, 0:1], in0=acc[:, 0:1], in1=rs[:])
                sq = scp.tile([C, QCH], F32, tag="sq")
                nc.scalar.square(out=sq[:, :qn], in_=tt[:, :qn])
                rq = scp.tile([C, 1], F32, tag="rq")
                nc.vector.tensor_reduce(out=rq[:], in_=sq[:, :qn], axis=AXX, op=ADD)
                nc.vector.tensor_add(out=acc[:, 1:2], in0=acc[:, 1:2], in1=rq[:])
                nc.sync.dma_start(
                    out=_dram_v(tt_out, c0, [[NPC, C], [1, qn]]), in_=tt[:, :qn])
            nc.sync.dma_start(out=stats, in_=acc[:])
    nc.compile()
    return nc


# --------------------------------------------------------------------------- L6
def _build_l6():
    nc = bacc.Bacc("TRN2", target_bir_lowering=False, debug=False, num_devices=1)
    ttin = nc.dram_tensor("ttin", [C, NPC], F32, kind="ExternalInput")
    wr2t = nc.dram_tensor("wr2t", [C, C], F32, kind="ExternalInput").ap()
    a3 = nc.dram_tensor("a3", [C, 1], F32, kind="ExternalInput").ap()
    d3 = nc.dram_tensor("d3", [C, 1], F32, kind="ExternalInput").ap()
    br2 = nc.dram_tensor("br2", [C, 1], F32, kind="ExternalInput").ap()
    outt = nc.dram_tensor("outt", [C, NPC], F32, kind="ExternalOutput")

    nq = -(-NPC // QCH)
    with tile.TileContext(nc) as tc:
        with (
            tc.tile_pool(name="u", bufs=3) as up,
            tc.tile_pool(name="o", bufs=3) as op,
            tc.tile_pool(name="cst", bufs=1) as cst,
            tc.tile_pool(name="ps", bufs=4, space="PSUM") as pp,
        ):
            w2 = cst.tile([C, C], F32)
            nc.sync.dma_start(out=w2[:], in_=wr2t)
            ca3 = cst.tile([C, 1], F32)
            nc.sync.dma_start(out=ca3[:], in_=a3)
            cd3 = cst.tile([C, 1], F32)
            nc.sync.dma_start(out=cd3[:], in_=d3)
            cbr = cst.tile([C, 1], F32)
            nc.sync.dma_start(out=cbr[:], in_=br2)
            for i in range(nq):
                c0 = i * QCH
                qn = min(QCH, NPC - c0)
                u = up.tile([C, QCH], F32)
                nc.sync.dma_start(
                    out=u[:, :qn], in_=_dram_v(ttin, c0, [[NPC, C], [1, qn]]))
                nc.scalar.activation(
                    out=u[:, :qn], in_=u[:, :qn],
                    func=mybir.ActivationFunctionType.Relu,
                    bias=cd3[:], scale=ca3[:])
                ps = pp.tile([C, QCH], F32)
                nc.tensor.matmul(ps[:, :qn], lhsT=w2[:], rhs=u[:, :qn],
                                 start=True, stop=True)
                ot = op.tile([C, QCH], F32)
                nc.vector.tensor_scalar_add(out=ot[:, :qn], in0=ps[:, :qn], scalar1=cbr[:])
                nc.sync.dma_start(
                    out=_dram_v(outt, c0, [[NPC, C], [1, qn]]), in_=ot[:, :qn])
    nc.compile()
    return nc


def _prog(name):
    if name not in _prog_cache:
        _prog_cache[name] = {
            "l1": _build_l1, "l2": _build_l2, "l3": _build_l3,
            "l4": _build_l4, "l5": _build_l5, "l6": _build_l6,
        }[name]()
    return _prog_cache[name]


def _run(name, in_maps):
    nc = _prog(name)
    res = run_bass_kernel_spmd(nc, in_maps, core_ids=list(range(NCORES)),
                               trace=PROFILE)
    if PROFILE:
        _last_exec_ns[name] = res.exec_time_ns
    return res.results


# ------------------------------------------------------------------------ host
def kernel(points, feature, index, W1, g1, b1, g2, b2, Wr1, br1, g3, b3, Wr2, br2):
    points = np.asarray(points, np.float32)
    feature = np.asarray(feature, np.float32)
    index = np.asarray(index)
    f32 = np.float32

    # ---- L1: G = X @ W1cat.T and P1 = points @ W1x.T, per-core rows ----
    w1cat_t = np.ascontiguousarray(np.asarray(W1, f32).T)          # [23, 20]
    w1x_t = np.zeros((CIN3, C), f32)
    w1x_t[C:, :] = w1cat_t[C:, :]
    wbd_g = np.zeros((CIN3 * MMB, C * MMB), f32)
    wbd_p = np.zeros((CIN3 * MMB, C * MMB), f32)
    for b in range(MMB):
        wbd_g[CIN3 * b:CIN3 * (b + 1), C * b:C * (b + 1)] = w1cat_t
        wbd_p[CIN3 * b:CIN3 * (b + 1), C * b:C * (b + 1)] = w1x_t

    in_maps = []
    for c in range(NCORES):
        sl = slice(c * NPC, (c + 1) * NPC)
        xt = np.zeros((CIN3, NPAD1), f32)
        xt[:C, :NPC] = feature[sl].T
        xt[C:, :NPC] = points[sl].T
        in_maps.append({"xt": xt, "wbd_g": wbd_g, "wbd_p": wbd_p})
    r1 = _run("l1", in_maps)

    g_full = np.zeros((N + 1, C), f32)
    p1 = np.zeros((NCORES, NPAD, C), f32)
    for c in range(NCORES):
        g_full[c * NPC:(c + 1) * NPC] = r1[c]["g_out"][:NPC]
        p1[c, :NPC] = r1[c]["p_out"][:NPC]
    p1t = p1.reshape(NCORES, NT, 128, CP * C)

    # ---- gather-layout index prep (int32, padded with zero-row index N) ----
    idxs = []
    for c in range(NCORES):
        ip = np.full((NPAD, KNN), N, np.int32)
        ip[:NPC] = index[c * NPC:(c + 1) * NPC].astype(np.int32)
        idxs.append(ip.reshape(NT, 128, SLOTS))

    # ---- L2: gather G, bn1 partial stats, dump fp16 ----
    in_maps = [{"table": g_full, "idx": idxs[c], "p1t": p1t[c]} for c in range(NCORES)]
    r2 = _run("l2", in_maps)

    sA = np.zeros(C, np.float64); sB = np.zeros(C, np.float64)
    sQ = np.zeros(C, np.float64); sP = np.zeros(C, np.float64)
    sPQ = np.zeros(C, np.float64)
    for c in range(NCORES):
        st = r2[c]["stats"].astype(np.float64)
        sA += st[:, 0:C].sum(0); sB += st[:, C:2 * C].sum(0)
        sQ += st[:, 2 * C:3 * C].sum(0); sP += st[:, 3 * C:4 * C].sum(0)
        sPQ += st[:, 4 * C:5 * C].sum(0)
    cnt = float(N * KNN)
    sx = sA - KNN * sP
    sxx = sQ - 2.0 * sB + KNN * sPQ
    m1 = sx / cnt
    v1 = sxx / cnt - m1 * m1
    a1 = np.asarray(g1, np.float64) / np.sqrt(v1 + EPS)
    d1 = np.asarray(b1, np.float64) - m1 * a1

    a1r = np.broadcast_to(a1.astype(f32), (128, C)).copy()
    asqr = np.broadcast_to((a1 * a1).astype(f32), (128, C)).copy()
    dbr = np.broadcast_to((d1 / a1).astype(f32), (128, C)).copy()

    # ---- L3: attention weights + first aggregation ----
    in_maps = [{"dump": r2[c]["dump"], "p1t": p1t[c], "a1r": a1r,
                "asqr": asqr, "dbr": dbr}
               for c in range(NCORES)]
    r3 = _run("l3", in_maps)

    x2_full = np.zeros((N + 1, C), np.float16)
    for c in range(NCORES):
        x2_full[c * NPC:(c + 1) * NPC] = \
            np.asarray(r3[c]["x2_out"]).reshape(NPAD, C)[:NPC]

    # ---- L4: gather x2, second aggregation, bn2 partial stats ----
    in_maps = [{"table": x2_full, "idx": idxs[c], "w_in": r3[c]["w_out"]}
               for c in range(NCORES)]
    r4 = _run("l4", in_maps)

    s3 = np.zeros(C, np.float64); s3q = np.zeros(C, np.float64)
    x3t_cores = []
    for c in range(NCORES):
        st = r4[c]["stats"].astype(np.float64)
        s3 += st[:, 0:C].sum(0); s3q += st[:, C:2 * C].sum(0)
        x3t_cores.append(np.ascontiguousarray(
            r4[c]["x3_out"].reshape(NPAD, C)[:NPC].T))
    m2 = s3 / N
    v2 = s3q / N - m2 * m2
    a2 = np.asarray(g2, np.float64) / np.sqrt(v2 + EPS)
    d2 = np.asarray(b2, np.float64) - m2 * a2

    # ---- L5: t = relu(bn2(x3)) ++ feature @ Wr1.T + br1, bn3 partial stats ----
    wr1t = np.ascontiguousarray(np.asarray(Wr1, f32).T)            # [40, 20]
    in_maps = []
    for c in range(NCORES):
        in_maps.append({
            "x3t": x3t_cores[c],
            "ft": np.ascontiguousarray(feature[c * NPC:(c + 1) * NPC].T),
            "wr1t": wr1t,
            "a2": a2.astype(f32).reshape(C, 1),
            "d2": d2.astype(f32).reshape(C, 1),
            "br1": np.asarray(br1, f32).reshape(C, 1),
        })
    r5 = _run("l5", in_maps)

    sT = np.zeros(C, np.float64); sTq = np.zeros(C, np.float64)
    for c in range(NCORES):
        st = r5[c]["stats"].astype(np.float64)
        sT += st[:, 0]; sTq += st[:, 1]
    m3 = sT / N
    v3 = sTq / N - m3 * m3
    a3 = np.asarray(g3, np.float64) / np.sqrt(v3 + EPS)
    d3 = np.asarray(b3, np.float64) - m3 * a3

    # ---- L6: out = relu(bn3(t)) @ Wr2.T + br2 ----
    wr2t = np.ascontiguousarray(np.asarray(Wr2, f32).T)            # [20, 20]
    in_maps = []
    for c in range(NCORES):
        in_maps.append({
            "ttin": r5[c]["tt_out"],
            "wr2t": wr2t,
            "a3": a3.astype(f32).reshape(C, 1),
            "d3": d3.astype(f32).reshape(C, 1),
            "br2": np.asarray(br2, f32).reshape(C, 1),
        })
    r6 = _run("l6", in_maps)

    out = np.empty((N, C), f32)
    for c in range(NCORES):
        out[c * NPC:(c + 1) * NPC] = r6[c]["outt"].T
    return out


# revision 16
# speedup vs baseline: 1.0047x; 1.0047x over previous
"""Trainium2 Bass kernel for nn_Attention2 (gnn message passing, N=1M, K=9, C=20).

Strategy: data-parallel over points (8 cores, 125k points each). KNN indices are
uniform-random global gathers; each core gathers from full replicated tables via
indirect DMA. Gather indices are batched GB=108 per indirect-DMA instruction
(the SWDGE fixed cost is ~1us per instruction, so per-row gathers are issue
bound; batched gathers are DMA-descriptor bound). The gathered pass-1 data is
dumped to DRAM in fp16 so the attention pass re-reads it sequentially instead
of re-gathering. Batch-norm statistics are computed as per-core partial sums
and combined on host (f64) between launches; all heavy math runs on device.

The attention math is refactored so the BN1 affine is applied via per-point
small tensors instead of full [N,K,C] ops:
  y[n,k,c]  = G[idx[n,k],c] - P1[n,c]          (G, P1 from L1 matmuls)
  xhat      = a1*y + d1 = a1 * u,  u = r - q', q' = P1 - d1/a1
  w[n,k]    = sum_c xhat[n,k,c]*xhat[n,0,c] = sum_c u[n,k,c] * (a1^2 u[n,0,c])
  x2[n,c]   = a1_c * sum_k u[n,k,c] w[n,k]
which needs only 3 full-size multiplies (DVE) + 2 full-size reduces (GpSimd).
"""

import sys

sys.path.insert(0, "/opt/trn_rl_repo")

import numpy as np

import concourse.bass as bass
import concourse.bacc as bacc
import concourse.tile as tile
from concourse import mybir
from concourse.bass_utils import run_bass_kernel_spmd

F32 = mybir.dt.float32
F16 = mybir.dt.float16
I32 = mybir.dt.int32
AXX = mybir.AxisListType.X
MUL = mybir.AluOpType.mult
ADD = mybir.AluOpType.add
SUB = mybir.AluOpType.subtract

KNN = 9
C = 20          # channels (in_channel == inner_channel == 20)
CIN3 = C + 3    # conv1 input channels
NCORES = 8
EPS = 1e-5
MMB = 5                    # row-blocks per L1 matmul (5 x 23 = 115 partitions)
L1ROWS = 128 * MMB         # rows per L1 iteration (640)
QCH = 512                  # column chunk for L5/L6 (matmul free dim <= 512)
CP = 48                    # points per partition per gather tile
PPT = 128 * CP             # points per gather tile
GB = 108                   # gather indices per indirect-DMA instruction
_prog_cache = {}


def _set_n(n):
    """Set problem size and derived constants (test hook; default N=1M)."""
    global N, NPC, NT, NPAD, SLOTS, FR, NL1, NPAD1
    N = n
    NPC = N // NCORES          # points per core
    NT = -(-NPC // PPT)        # gather tiles per core
    NPAD = NT * PPT            # padded points per core (gather layout)
    SLOTS = CP * KNN           # gather slots per partition per tile (432)
    FR = SLOTS * C             # gathered floats per partition per tile (8640)
    NL1 = -(-NPC // L1ROWS)    # L1 iterations
    NPAD1 = NL1 * L1ROWS
    _prog_cache.clear()



PROFILE = False            # test.py sets this; grader path keeps False
_last_exec_ns = {}


_set_n(1_000_000)


def _v(t_ap, dims, extra_offset=0):
    """View a tile/tensor AP with explicit free-dim (step, count) pairs."""
    return bass.AP(
        tensor=t_ap.tensor,
        offset=t_ap.offset + extra_offset,
        ap=[t_ap.ap[0]] + [[s, c] for s, c in dims],
    )


def _dram_v(t, offset, dims):
    """Arbitrary affine view of a DRAM tensor (element offset, [step,count] dims)."""
    return bass.AP(tensor=t, offset=offset, ap=[[s, c] for s, c in dims])


def _gather(nc, r, table, it):
    """Gather SLOTS rows of C elements per partition from table. The HW
    consumes exactly one index per partition per indirect-DMA instruction
    (measured: multi-index offset APs silently use only idx[p,0]), so this
    is SLOTS instructions; each costs ~1.4us of Pool-engine descriptor
    generation, which is the kernel's overall bottleneck."""
    for s in range(SLOTS):
        nc.gpsimd.indirect_dma_start(
            out=r[:, s * C:(s + 1) * C],
            out_offset=None,
            in_=table,
            in_offset=bass.IndirectOffsetOnAxis(ap=it[:, s:s + 1], axis=0),
        )


# --------------------------------------------------------------------------- L1
def _build_l1():
    nc = bacc.Bacc("TRN2", target_bir_lowering=False, debug=False, num_devices=1)
    xt = nc.dram_tensor("xt", [CIN3, NPAD1], F32, kind="ExternalInput")
    wbd_g = nc.dram_tensor("wbd_g", [CIN3 * MMB, C * MMB], F32, kind="ExternalInput").ap()
    wbd_p = nc.dram_tensor("wbd_p", [CIN3 * MMB, C * MMB], F32, kind="ExternalInput").ap()
    g_out = nc.dram_tensor("g_out", [NPAD1, C], F32, kind="ExternalOutput")
    p_out = nc.dram_tensor("p_out", [NPAD1, C], F32, kind="ExternalOutput")

    KP = CIN3 * MMB  # 115
    with tile.TileContext(nc) as tc:
        with (
            tc.tile_pool(name="w", bufs=1) as wp,
            tc.tile_pool(name="x", bufs=3) as xp,
            tc.tile_pool(name="o", bufs=3) as op,
            tc.tile_pool(name="ps", bufs=4, space="PSUM") as pp,
        ):
            wg = wp.tile([KP, C * MMB], F32)
            nc.sync.dma_start(out=wg[:], in_=wbd_g)
            wpt = wp.tile([KP, C * MMB], F32)
            nc.sync.dma_start(out=wpt[:], in_=wbd_p)
            for i in range(NL1):
                r0 = i * L1ROWS
                lhs = xp.tile([KP, 128], F32)
                nc.sync.dma_start(
                    out=lhs[:],
                    in_=_dram_v(xt, r0, [[128, MMB], [NPAD1, CIN3], [1, 128]]),
                )
                psg = pp.tile([128, C * MMB], F32, tag="psg")
                nc.tensor.matmul(psg[:], lhsT=lhs[:], rhs=wg[:], start=True, stop=True)
                psp = pp.tile([128, C * MMB], F32, tag="psp")
                nc.tensor.matmul(psp[:], lhsT=lhs[:], rhs=wpt[:], start=True, stop=True)
                og = op.tile([128, C * MMB], F32, tag="og")
                nc.scalar.copy(out=og[:], in_=psg[:])
                opt = op.tile([128, C * MMB], F32, tag="opt")
                nc.scalar.copy(out=opt[:], in_=psp[:])
                dst = [[C, 128], [128 * C, MMB], [1, C]]
                nc.sync.dma_start(out=_dram_v(g_out, r0 * C, dst), in_=og[:])
                nc.sync.dma_start(out=_dram_v(p_out, r0 * C, dst), in_=opt[:])
    nc.compile()
    return nc


# --------------------------------------------------------------------------- L2
def _build_l2():
    nc = bacc.Bacc("TRN2", target_bir_lowering=False, debug=False, num_devices=1)
    table = nc.dram_tensor("table", [N + 1, C], F32, kind="ExternalInput").ap()
    idx = nc.dram_tensor("idx", [NT, 128, SLOTS], I32, kind="ExternalInput").ap()
    p1t = nc.dram_tensor("p1t", [NT, 128, CP * C], F32, kind="ExternalInput").ap()
    dump = nc.dram_tensor("dump", [NT, 128, FR], F16, kind="ExternalOutput").ap()
    stats = nc.dram_tensor("stats", [128, 5 * C], F32, kind="ExternalOutput").ap()

    with tile.TileContext(nc) as tc:
        with (
            tc.tile_pool(name="g", bufs=2) as gp,
            tc.tile_pool(name="h", bufs=2) as hp,
            tc.tile_pool(name="i", bufs=2) as ip,
            tc.tile_pool(name="p1", bufs=2) as p1p,
            tc.tile_pool(name="sc", bufs=2) as scp,
            tc.tile_pool(name="acc", bufs=1) as accp,
        ):
            acc = accp.tile([128, 5 * C], F32)
            nc.vector.memset(acc[:], 0.0)
            for t in range(NT):
                it = ip.tile([128, SLOTS], I32)
                nc.sync.dma_start(out=it[:], in_=idx[t])
                p1 = p1p.tile([128, CP * C], F32)
                nc.sync.dma_start(out=p1[:], in_=p1t[t])
                r = gp.tile([128, FR], F32)
                _gather(nc, r, table, it)
                # fp16 copy of the gathered rows for the dump (ACT engine)
                r16 = hp.tile([128, FR], F16)
                nc.scalar.copy(out=r16[:], in_=r[:])
                # S[j,c] = sum_k r[j,k,c]
                sv = scp.tile([128, CP * C], F32, tag="sv")
                nc.vector.tensor_reduce(
                    out=sv[:], in_=_v(r[:], [[KNN * C, CP], [1, C], [C, KNN]]),
                    axis=AXX, op=ADD)
                # accA += sum_j S
                rA = scp.tile([128, C], F32, tag="rA")
                nc.vector.tensor_reduce(
                    out=rA[:], in_=_v(sv[:], [[1, C], [C, CP]]), axis=AXX, op=ADD)
                nc.vector.tensor_add(out=acc[:, 0:C], in0=acc[:, 0:C], in1=rA[:])
                # accB += sum_j P1*S
                ps = scp.tile([128, CP * C], F32, tag="ps")
                nc.vector.tensor_tensor(out=ps[:], in0=p1[:], in1=sv[:], op=MUL)
                rB = scp.tile([128, C], F32, tag="rB")
                nc.vector.tensor_reduce(
                    out=rB[:], in_=_v(ps[:], [[1, C], [C, CP]]), axis=AXX, op=ADD)
                nc.vector.tensor_add(out=acc[:, C:2 * C], in0=acc[:, C:2 * C], in1=rB[:])
                # accQ += sum_{j,k} r^2
                sq = scp.tile([128, FR], F32, tag="sq")
                nc.scalar.square(out=sq[:], in_=r[:])
                rQ = scp.tile([128, C], F32, tag="rQ")
                nc.vector.tensor_reduce(
                    out=rQ[:], in_=_v(sq[:], [[1, C], [C, SLOTS]]), axis=AXX, op=ADD)
                nc.vector.tensor_add(out=acc[:, 2 * C:3 * C], in0=acc[:, 2 * C:3 * C], in1=rQ[:])
                # accP += sum_j P1 ; accPQ += sum_j P1^2
                rP = scp.tile([128, C], F32, tag="rP")
                nc.vector.tensor_reduce(
                    out=rP[:], in_=_v(p1[:], [[1, C], [C, CP]]), axis=AXX, op=ADD)
                nc.vector.tensor_add(out=acc[:, 3 * C:4 * C], in0=acc[:, 3 * C:4 * C], in1=rP[:])
                p1s = scp.tile([128, CP * C], F32, tag="p1s")
                nc.scalar.square(out=p1s[:], in_=p1[:])
                rPQ = scp.tile([128, C], F32, tag="rPQ")
                nc.vector.tensor_reduce(
                    out=rPQ[:], in_=_v(p1s[:], [[1, C], [C, CP]]), axis=AXX, op=ADD)
                nc.vector.tensor_add(out=acc[:, 4 * C:5 * C], in0=acc[:, 4 * C:5 * C], in1=rPQ[:])
                nc.sync.dma_start(out=dump[t], in_=r16[:])
            nc.sync.dma_start(out=stats, in_=acc[:])
    nc.compile()
    return nc


# --------------------------------------------------------------------------- L3
def _build_l3():
    nc = bacc.Bacc("TRN2", target_bir_lowering=False, debug=False, num_devices=1)
    dump = nc.dram_tensor("dump", [NT, 128, FR], F16, kind="ExternalInput").ap()
    p1t = nc.dram_tensor("p1t", [NT, 128, CP * C], F32, kind="ExternalInput").ap()
    a1r = nc.dram_tensor("a1r", [128, C], F32, kind="ExternalInput").ap()
    asqr = nc.dram_tensor("asqr", [128, C], F32, kind="ExternalInput").ap()
    dbr = nc.dram_tensor("dbr", [128, C], F32, kind="ExternalInput").ap()
    w_out = nc.dram_tensor("w_out", [NT, 128, SLOTS], F16, kind="ExternalOutput").ap()
    x2_out = nc.dram_tensor("x2_out", [NT, 128, CP * C], F16, kind="ExternalOutput").ap()

    with tile.TileContext(nc) as tc:
        with (
            tc.tile_pool(name="g", bufs=2) as gp,
            tc.tile_pool(name="big", bufs=2) as bigp,
            tc.tile_pool(name="p1", bufs=2) as p1p,
            tc.tile_pool(name="sc", bufs=2) as scp,
            tc.tile_pool(name="cst", bufs=1) as cst,
        ):
            a1 = cst.tile([128, C], F32)
            nc.sync.dma_start(out=a1[:], in_=a1r)
            asq = cst.tile([128, C], F32)
            nc.sync.dma_start(out=asq[:], in_=asqr)
            db = cst.tile([128, C], F32)
            nc.sync.dma_start(out=db[:], in_=dbr)

            # Two-stage software pipeline so the Pool w-reduce of tile t
            # overlaps the DVE tail (pw2/s/x2) of tile t-1.
            st = {}

            def stage_a(t):
                r16 = gp.tile([128, FR], F16, tag="r16")
                nc.sync.dma_start(out=r16[:], in_=dump[t])
                p1 = p1p.tile([128, CP * C], F32)
                nc.sync.dma_start(out=p1[:], in_=p1t[t])
                # q'[j,c] = P1 - d1/a1   (per point j, channel c)
                qp = scp.tile([128, CP * C], F16, tag="qp")
                nc.vector.tensor_tensor(
                    out=qp[:], in0=p1[:],
                    in1=_v(db[:], [[0, CP], [1, C]]), op=SUB)
                # u = r - q'
                u = bigp.tile([128, FR], F16, tag="u")
                nc.vector.tensor_tensor(
                    out=u[:], in0=r16[:],
                    in1=_v(qp[:], [[C, CP], [0, KNN], [1, C]]), op=SUB)
                # z[j,c] = a1^2 * u[j,0,c]
                z = scp.tile([128, CP * C], F16, tag="z")
                nc.vector.tensor_tensor(
                    out=z[:], in0=_v(u[:], [[KNN * C, CP], [1, C]]),
                    in1=_v(asq[:], [[0, CP], [1, C]]), op=MUL)
                # w[j,k] = sum_c u[j,k,c]*z[j,c]
                pw = bigp.tile([128, FR], F16, tag="pw")
                nc.vector.tensor_tensor(
                    out=pw[:], in0=u[:],
                    in1=_v(z[:], [[C, CP], [0, KNN], [1, C]]), op=MUL)
                w = scp.tile([128, SLOTS], F16, tag="w")
                with nc.allow_low_precision(reason="20-term fp16 dot, 2e-2 tol"):
                    nc.vector.tensor_reduce(
                        out=w[:], in_=_v(pw[:], [[KNN * C, CP], [C, KNN], [1, C]]),
                        axis=AXX, op=ADD)
                st[t] = (u, w)

            def stage_b(t):
                u, w = st.pop(t)
                # s[j,c] = sum_k u[j,k,c]*w[j,k] ; x2 = a1*s
                pw2 = bigp.tile([128, FR], F16, tag="pw2")
                nc.vector.tensor_tensor(
                    out=pw2[:], in0=u[:],
                    in1=_v(w[:], [[KNN, CP], [1, KNN], [0, C]]), op=MUL)
                s = scp.tile([128, CP * C], F16, tag="s")
                with nc.allow_low_precision(reason="9-term fp16 sum, 2e-2 tol"):
                    nc.vector.tensor_reduce(
                        out=s[:], in_=_v(pw2[:], [[KNN * C, CP], [1, C], [C, KNN]]),
                        axis=AXX, op=ADD)
                x2 = scp.tile([128, CP * C], F16, tag="x2")
                nc.vector.tensor_tensor(
                    out=x2[:], in0=s[:],
                    in1=_v(a1[:], [[0, CP], [1, C]]), op=MUL)
                nc.sync.dma_start(out=w_out[t], in_=w[:])
                nc.sync.dma_start(out=x2_out[t], in_=x2[:])

            stage_a(0)
            for t in range(1, NT):
                stage_a(t)
                stage_b(t - 1)
            stage_b(NT - 1)
    nc.compile()
    return nc


# --------------------------------------------------------------------------- L4
def _build_l4():
    nc = bacc.Bacc("TRN2", target_bir_lowering=False, debug=False, num_devices=1)
    table = nc.dram_tensor("table", [N + 1, C], F16, kind="ExternalInput").ap()
    idx = nc.dram_tensor("idx", [NT, 128, SLOTS], I32, kind="ExternalInput").ap()
    w_in = nc.dram_tensor("w_in", [NT, 128, SLOTS], F16, kind="ExternalInput").ap()
    x3_out = nc.dram_tensor("x3_out", [NT, 128, CP * C], F32, kind="ExternalOutput").ap()
    stats = nc.dram_tensor("stats", [128, 2 * C], F32, kind="ExternalOutput").ap()

    with tile.TileContext(nc) as tc:
        with (
            tc.tile_pool(name="g", bufs=2) as gp,
            tc.tile_pool(name="i", bufs=2) as ip,
            tc.tile_pool(name="w", bufs=2) as wp,
            tc.tile_pool(name="px", bufs=2) as pxp,
            tc.tile_pool(name="sc", bufs=2) as scp,
            tc.tile_pool(name="acc", bufs=1) as accp,
        ):
            acc = accp.tile([128, 2 * C], F32)
            nc.vector.memset(acc[:], 0.0)
            for t in range(NT):
                it = ip.tile([128, SLOTS], I32)
                nc.sync.dma_start(out=it[:], in_=idx[t])
                wt = wp.tile([128, SLOTS], F16)
                nc.sync.dma_start(out=wt[:], in_=w_in[t])
                r = gp.tile([128, FR], F16)
                _gather(nc, r, table, it)
                # px = r * w  (broadcast over c; f32 — the product tail can
                # exceed fp16 range)
                px = pxp.tile([128, FR], F32, tag="px")
                nc.vector.tensor_tensor(
                    out=px[:], in0=r[:],
                    in1=_v(wt[:], [[KNN, CP], [1, KNN], [0, C]]), op=MUL)
                x3 = scp.tile([128, CP * C], F32, tag="x3")
                nc.vector.tensor_reduce(
                    out=x3[:], in_=_v(px[:], [[KNN * C, CP], [1, C], [C, KNN]]),
                    axis=AXX, op=ADD)
                rA = scp.tile([128, C], F32, tag="rA")
                nc.vector.tensor_reduce(
                    out=rA[:], in_=_v(x3[:], [[1, C], [C, CP]]), axis=AXX, op=ADD)
                nc.vector.tensor_add(out=acc[:, 0:C], in0=acc[:, 0:C], in1=rA[:])
                sq = scp.tile([128, CP * C], F32, tag="sq")
                nc.scalar.square(out=sq[:], in_=x3[:])
                rB = scp.tile([128, C], F32, tag="rB")
                nc.vector.tensor_reduce(
                    out=rB[:], in_=_v(sq[:], [[1, C], [C, CP]]), axis=AXX, op=ADD)
                nc.vector.tensor_add(out=acc[:, C:2 * C], in0=acc[:, C:2 * C], in1=rB[:])
                nc.sync.dma_start(out=x3_out[t], in_=x3[:])
            nc.sync.dma_start(out=stats, in_=acc[:])
    nc.compile()
    return nc


# --------------------------------------------------------------------------- L5
def _build_l5():
    nc = bacc.Bacc("TRN2", target_bir_lowering=False, debug=False, num_devices=1)
    x3t = nc.dram_tensor("x3t", [C, NPC], F32, kind="ExternalInput")
    ft = nc.dram_tensor("ft", [C, NPC], F32, kind="ExternalInput")
    wr1t = nc.dram_tensor("wr1t", [2 * C, C], F32, kind="ExternalInput").ap()
    a2 = nc.dram_tensor("a2", [C, 1], F32, kind="ExternalInput").ap()
    d2 = nc.dram_tensor("d2", [C, 1], F32, kind="ExternalInput").ap()
    br1 = nc.dram_tensor("br1", [C, 1], F32, kind="ExternalInput").ap()
    tt_out = nc.dram_tensor("tt_out", [C, NPC], F32, kind="ExternalOutput")
    stats = nc.dram_tensor("stats", [C, 2], F32, kind="ExternalOutput").ap()

    nq = -(-NPC // QCH)
    with tile.TileContext(nc) as tc:
        with (
            tc.tile_pool(name="u", bufs=3) as up,
            tc.tile_pool(name="o", bufs=3) as op,
            tc.tile_pool(name="sc", bufs=3) as scp,
            tc.tile_pool(name="cst", bufs=1) as cst,
            tc.tile_pool(name="acc", bufs=1) as accp,
            tc.tile_pool(name="ps", bufs=4, space="PSUM") as pp,
        ):
            w1 = cst.tile([2 * C, C], F32)
            nc.sync.dma_start(out=w1[:], in_=wr1t)
            ca2 = cst.tile([C, 1], F32)
            nc.sync.dma_start(out=ca2[:], in_=a2)
            cd2 = cst.tile([C, 1], F32)
            nc.sync.dma_start(out=cd2[:], in_=d2)
            cbr = cst.tile([C, 1], F32)
            nc.sync.dma_start(out=cbr[:], in_=br1)
            acc = accp.tile([C, 2], F32)
            nc.vector.memset(acc[:], 0.0)
            for i in range(nq):
                c0 = i * QCH
                qn = min(QCH, NPC - c0)
                u = up.tile([2 * C, QCH], F32)
                nc.sync.dma_start(
                    out=u[0:C, :qn], in_=_dram_v(x3t, c0, [[NPC, C], [1, qn]]))
                nc.sync.dma_start(
                    out=u[C:2 * C, :qn], in_=_dram_v(ft, c0, [[NPC, C], [1, qn]]))
                nc.scalar.activation(
                    out=u[0:C, :qn], in_=u[0:C, :qn],
                    func=mybir.ActivationFunctionType.Relu,
                    bias=cd2[:], scale=ca2[:])
                ps = pp.tile([C, QCH], F32)
                nc.tensor.matmul(ps[:, :qn], lhsT=w1[:], rhs=u[:, :qn],
                                 start=True, stop=True)
                tt = op.tile([C, QCH], F32)
                nc.vector.tensor_scalar_add(out=tt[:, :qn], in0=ps[:, :qn], scalar1=cbr[:])
                rs = scp.tile([C, 1], F32, tag="rs")
                nc.vector.tensor_reduce(out=rs[:], in_=tt[:, :qn], axis=AXX, op=ADD)
                nc.vector.tensor_add(out=acc[:, 0:1], in0=acc[:, 0:1], in1=rs[:])
                sq = scp.tile([C, QCH], F32, tag="sq")
                nc.scalar.square(out=sq[:, :qn], in_=tt[:, :qn])
                rq = scp.tile([C, 1], F32, tag="rq")
                nc.vector.tensor_reduce(out=rq[:], in_=sq[:, :qn], axis=AXX, op=ADD)
                nc.vector.tensor_add(out=acc[:, 1:2], in0=acc[:, 1:2], in1=rq[:])
                nc.sync.dma_start(
                    out=_dram_v(tt_out, c0, [[NPC, C], [1, qn]]), in_=tt[:, :qn])
            nc.sync.dma_start(out=stats, in_=acc[:])
    nc.compile()
    return nc


# --------------------------------------------------------------------------- L6
def _build_l6():
    nc = bacc.Bacc("TRN2", target_bir_lowering=False, debug=False, num_devices=1)
    ttin = nc.dram_tensor("ttin", [C, NPC], F32, kind="ExternalInput")
    wr2t = nc.dram_tensor("wr2t", [C, C], F32, kind="ExternalInput").ap()
    a3 = nc.dram_tensor("a3", [C, 1], F32, kind="ExternalInput").ap()
    d3 = nc.dram_tensor("d3", [C, 1], F32, kind="ExternalInput").ap()
    br2 = nc.dram_tensor("br2", [C, 1], F32, kind="ExternalInput").ap()
    outt = nc.dram_tensor("outt", [C, NPC], F32, kind="ExternalOutput")

    nq = -(-NPC // QCH)
    with tile.TileContext(nc) as tc:
        with (
            tc.tile_pool(name="u", bufs=3) as up,
            tc.tile_pool(name="o", bufs=3) as op,
            tc.tile_pool(name="cst", bufs=1) as cst,
            tc.tile_pool(name="ps", bufs=4, space="PSUM") as pp,
        ):
            w2 = cst.tile([C, C], F32)
            nc.sync.dma_start(out=w2[:], in_=wr2t)
            ca3 = cst.tile([C, 1], F32)
            nc.sync.dma_start(out=ca3[:], in_=a3)
            cd3 = cst.tile([C, 1], F32)
            nc.sync.dma_start(out=cd3[:], in_=d3)
            cbr = cst.tile([C, 1], F32)
            nc.sync.dma_start(out=cbr[:], in_=br2)
            for i in range(nq):
                c0 = i * QCH
                qn = min(QCH, NPC - c0)
                u = up.tile([C, QCH], F32)
                nc.sync.dma_start(
                    out=u[:, :qn], in_=_dram_v(ttin, c0, [[NPC, C], [1, qn]]))
                nc.scalar.activation(
                    out=u[:, :qn], in_=u[:, :qn],
                    func=mybir.ActivationFunctionType.Relu,
                    bias=cd3[:], scale=ca3[:])
                ps = pp.tile([C, QCH], F32)
                nc.tensor.matmul(ps[:, :qn], lhsT=w2[:], rhs=u[:, :qn],
                                 start=True, stop=True)
                ot = op.tile([C, QCH], F32)
                nc.vector.tensor_scalar_add(out=ot[:, :qn], in0=ps[:, :qn], scalar1=cbr[:])
                nc.sync.dma_start(
                    out=_dram_v(outt, c0, [[NPC, C], [1, qn]]), in_=ot[:, :qn])
    nc.compile()
    return nc


def _prog(name):
    if name not in _prog_cache:
        _prog_cache[name] = {
            "l1": _build_l1, "l2": _build_l2, "l3": _build_l3,
            "l4": _build_l4, "l5": _build_l5, "l6": _build_l6,
        }[name]()
    return _prog_cache[name]


def _run(name, in_maps):
    nc = _prog(name)
    res = run_bass_kernel_spmd(nc, in_maps, core_ids=list(range(NCORES)),
                               trace=PROFILE)
    if PROFILE:
        _last_exec_ns[name] = res.exec_time_ns
    return res.results


# ------------------------------------------------------------------------ host
def kernel(points, feature, index, W1, g1, b1, g2, b2, Wr1, br1, g3, b3, Wr2, br2):
    points = np.asarray(points, np.float32)
    feature = np.asarray(feature, np.float32)
    index = np.asarray(index)
    f32 = np.float32

    # ---- L1: G = X @ W1cat.T and P1 = points @ W1x.T, per-core rows ----
    w1cat_t = np.ascontiguousarray(np.asarray(W1, f32).T)          # [23, 20]
    w1x_t = np.zeros((CIN3, C), f32)
    w1x_t[C:, :] = w1cat_t[C:, :]
    wbd_g = np.zeros((CIN3 * MMB, C * MMB), f32)
    wbd_p = np.zeros((CIN3 * MMB, C * MMB), f32)
    for b in range(MMB):
        wbd_g[CIN3 * b:CIN3 * (b + 1), C * b:C * (b + 1)] = w1cat_t
        wbd_p[CIN3 * b:CIN3 * (b + 1), C * b:C * (b + 1)] = w1x_t

    in_maps = []
    for c in range(NCORES):
        sl = slice(c * NPC, (c + 1) * NPC)
        xt = np.zeros((CIN3, NPAD1), f32)
        xt[:C, :NPC] = feature[sl].T
        xt[C:, :NPC] = points[sl].T
        in_maps.append({"xt": xt, "wbd_g": wbd_g, "wbd_p": wbd_p})
    r1 = _run("l1", in_maps)

    g_full = np.zeros((N + 1, C), f32)
    p1 = np.zeros((NCORES, NPAD, C), f32)
    for c in range(NCORES):
        g_full[c * NPC:(c + 1) * NPC] = r1[c]["g_out"][:NPC]
        p1[c, :NPC] = r1[c]["p_out"][:NPC]
    p1t = p1.reshape(NCORES, NT, 128, CP * C)

    # ---- gather-layout index prep (int32, padded with zero-row index N) ----
    idxs = []
    for c in range(NCORES):
        ip = np.full((NPAD, KNN), N, np.int32)
        ip[:NPC] = index[c * NPC:(c + 1) * NPC].astype(np.int32)
        idxs.append(ip.reshape(NT, 128, SLOTS))

    # ---- L2: gather G, bn1 partial stats, dump fp16 ----
    in_maps = [{"table": g_full, "idx": idxs[c], "p1t": p1t[c]} for c in range(NCORES)]
    r2 = _run("l2", in_maps)

    sA = np.zeros(C, np.float64); sB = np.zeros(C, np.float64)
    sQ = np.zeros(C, np.float64); sP = np.zeros(C, np.float64)
    sPQ = np.zeros(C, np.float64)
    for c in range(NCORES):
        st = r2[c]["stats"].astype(np.float64)
        sA += st[:, 0:C].sum(0); sB += st[:, C:2 * C].sum(0)
        sQ += st[:, 2 * C:3 * C].sum(0); sP += st[:, 3 * C:4 * C].sum(0)
        sPQ += st[:, 4 * C:5 * C].sum(0)
    cnt = float(N * KNN)
    sx = sA - KNN * sP
    sxx = sQ - 2.0 * sB + KNN * sPQ
    m1 = sx / cnt
    v1 = sxx / cnt - m1 * m1
    a1 = np.asarray(g1, np.float64) / np.sqrt(v1 + EPS)
    d1 = np.asarray(b1, np.float64) - m1 * a1

    a1r = np.broadcast_to(a1.astype(f32), (128, C)).copy()
    asqr = np.broadcast_to((a1 * a1).astype(f32), (128, C)).copy()
    dbr = np.broadcast_to((d1 / a1).astype(f32), (128, C)).copy()

    # ---- L3: attention weights + first aggregation ----
    in_maps = [{"dump": r2[c]["dump"], "p1t": p1t[c], "a1r": a1r,
                "asqr": asqr, "dbr": dbr}
               for c in range(NCORES)]
    r3 = _run("l3", in_maps)

    x2_full = np.zeros((N + 1, C), np.float16)
    for c in range(NCORES):
        x2_full[c * NPC:(c + 1) * NPC] = \
            np.asarray(r3[c]["x2_out"]).reshape(NPAD, C)[:NPC]

    # ---- L4: gather x2, second aggregation, bn2 partial stats ----
    in_maps = [{"table": x2_full, "idx": idxs[c], "w_in": r3[c]["w_out"]}
               for c in range(NCORES)]
    r4 = _run("l4", in_maps)

    s3 = np.zeros(C, np.float64); s3q = np.zeros(C, np.float64)
    x3t_cores = []
    for c in range(NCORES):
        st = r4[c]["stats"].astype(np.float64)
        s3 += st[:, 0:C].sum(0); s3q += st[:, C:2 * C].sum(0)
        x3t_cores.append(np.ascontiguousarray(
            r4[c]["x3_out"].reshape(NPAD, C)[:NPC].T))
    m2 = s3 / N
    v2 = s3q / N - m2 * m2
    a2 = np.asarray(g2, np.float64) / np.sqrt(v2 + EPS)
    d2 = np.asarray(b2, np.float64) - m2 * a2

    # ---- L5: t = relu(bn2(x3)) ++ feature @ Wr1.T + br1, bn3 partial stats ----
    wr1t = np.ascontiguousarray(np.asarray(Wr1, f32).T)            # [40, 20]
    in_maps = []
    for c in range(NCORES):
        in_maps.append({
            "x3t": x3t_cores[c],
            "ft": np.ascontiguousarray(feature[c * NPC:(c + 1) * NPC].T),
            "wr1t": wr1t,
            "a2": a2.astype(f32).reshape(C, 1),
            "d2": d2.astype(f32).reshape(C, 1),
            "br1": np.asarray(br1, f32).reshape(C, 1),
        })
    r5 = _run("l5", in_maps)

    sT = np.zeros(C, np.float64); sTq = np.zeros(C, np.float64)
    for c in range(NCORES):
        st = r5[c]["stats"].astype(np.float64)
        sT += st[:, 0]; sTq += st[:, 1]
    m3 = sT / N
    v3 = sTq / N - m3 * m3
    a3 = np.asarray(g3, np.float64) / np.sqrt(v3 + EPS)
    d3 = np.asarray(b3, np.float64) - m3 * a3

    # ---- L6: out = relu(bn3(t)) @ Wr2.T + br2 ----
    wr2t = np.ascontiguousarray(np.asarray(Wr2, f32).T)            # [20, 20]
    in_maps = []
    for c in range(NCORES):
        in_maps.append({
            "ttin": r5[c]["tt_out"],
            "wr2t": wr2t,
            "a3": a3.astype(f32).reshape(C, 1),
            "d3": d3.astype(f32).reshape(C, 1),
            "br2": np.asarray(br2, f32).reshape(C, 1),
        })
    r6 = _run("l6", in_maps)

    out = np.empty((N, C), f32)
    for c in range(NCORES):
        out[c * NPC:(c + 1) * NPC] = r6[c]["outt"].T
    return out


# revision 21
# speedup vs baseline: 1.0420x; 1.0371x over previous
"""Trainium2 Bass kernel for nn_Attention2 (gnn message passing, N=1M, K=9, C=20).

Strategy: data-parallel over points (8 cores, 125k points each). KNN indices are
uniform-random global gathers; each core gathers from full replicated tables via
indirect DMA, 128 rows (one per partition) per instruction — the HW consumes
exactly one offset index per partition, and the ~1.4us/instruction SWDGE
descriptor-generation floor on the Pool engine is the kernel's bottleneck
(2 x 9072 instructions/core ~= 26ms of the ~29ms total). The gathered pass-1
data is dumped to DRAM in fp16 so the attention pass re-reads it sequentially
instead of re-gathering. Batch-norm statistics are computed as per-core partial
sums and combined on host (f64) between launches; all heavy math runs on
device.

The attention math is refactored so the BN1 affine is applied via per-point
small tensors instead of full [N,K,C] ops:
  y[n,k,c]  = G[idx[n,k],c] - P1[n,c]          (G, P1 from L1 matmuls)
  xhat      = a1*y + d1 = a1 * u,  u = r - q', q' = P1 - d1/a1
  w[n,k]    = sum_c xhat[n,k,c]*xhat[n,0,c] = sum_c u[n,k,c] * (a1^2 u[n,0,c])
  x2[n,c]   = a1_c * sum_k u[n,k,c] w[n,k]
which needs only 3 full-size multiplies (DVE) + 2 full-size reduces (GpSimd).
"""

import sys

sys.path.insert(0, "/opt/trn_rl_repo")

import numpy as np

import concourse.bass as bass
import concourse.bacc as bacc
import concourse.tile as tile
from concourse import mybir
from concourse.bass_utils import run_bass_kernel_spmd

F32 = mybir.dt.float32
F16 = mybir.dt.float16
I32 = mybir.dt.int32
AXX = mybir.AxisListType.X
MUL = mybir.AluOpType.mult
ADD = mybir.AluOpType.add
SUB = mybir.AluOpType.subtract

KNN = 9
C = 20          # channels (in_channel == inner_channel == 20)
CIN3 = C + 3    # conv1 input channels
NCORES = 8
EPS = 1e-5
MMB = 5                    # row-blocks per L1 matmul (5 x 23 = 115 partitions)
L1ROWS = 128 * MMB         # rows per L1 iteration (640)
QCH = 512                  # column chunk for L5/L6 (matmul free dim <= 512)
CP = 49                    # points per partition per gather tile (NT=20,
                           # NPAD=125440: only 0.35% padded gather slots)
PPT = 128 * CP             # points per gather tile
_prog_cache = {}


def _set_n(n):
    """Set problem size and derived constants (test hook; default N=1M)."""
    global N, NPC, NT, NPAD, SLOTS, FR, NL1, NPAD1
    N = n
    NPC = N // NCORES          # points per core
    NT = -(-NPC // PPT)        # gather tiles per core
    NPAD = NT * PPT            # padded points per core (gather layout)
    SLOTS = CP * KNN           # gather slots per partition per tile (432)
    FR = SLOTS * C             # gathered floats per partition per tile (8640)
    NL1 = -(-NPC // L1ROWS)    # L1 iterations
    NPAD1 = NL1 * L1ROWS
    _prog_cache.clear()



PROFILE = False            # test.py sets this; grader path keeps False
_last_exec_ns = {}


_set_n(1_000_000)


def _v(t_ap, dims, extra_offset=0):
    """View a tile/tensor AP with explicit free-dim (step, count) pairs."""
    return bass.AP(
        tensor=t_ap.tensor,
        offset=t_ap.offset + extra_offset,
        ap=[t_ap.ap[0]] + [[s, c] for s, c in dims],
    )


def _dram_v(t, offset, dims):
    """Arbitrary affine view of a DRAM tensor (element offset, [step,count] dims)."""
    return bass.AP(tensor=t, offset=offset, ap=[[s, c] for s, c in dims])


def _gather(nc, r, table, it):
    """Gather SLOTS rows of C elements per partition from table. The HW
    consumes exactly one index per partition per indirect-DMA instruction
    (measured: multi-index offset APs silently use only idx[p,0]), so this
    is SLOTS instructions; each costs ~1.4us of Pool-engine descriptor
    generation, which is the kernel's overall bottleneck."""
    for s in range(SLOTS):
        nc.gpsimd.indirect_dma_start(
            out=r[:, s * C:(s + 1) * C],
            out_offset=None,
            in_=table,
            in_offset=bass.IndirectOffsetOnAxis(ap=it[:, s:s + 1], axis=0),
        )


# --------------------------------------------------------------------------- L1
def _build_l1():
    nc = bacc.Bacc("TRN2", target_bir_lowering=False, debug=False, num_devices=1)
    xt = nc.dram_tensor("xt", [CIN3, NPAD1], F32, kind="ExternalInput")
    wbd_g = nc.dram_tensor("wbd_g", [CIN3 * MMB, C * MMB], F32, kind="ExternalInput").ap()
    wbd_p = nc.dram_tensor("wbd_p", [CIN3 * MMB, C * MMB], F32, kind="ExternalInput").ap()
    g_out = nc.dram_tensor("g_out", [NPAD1, C], F32, kind="ExternalOutput")
    p_out = nc.dram_tensor("p_out", [NPAD1, C], F32, kind="ExternalOutput")

    KP = CIN3 * MMB  # 115
    with tile.TileContext(nc) as tc:
        with (
            tc.tile_pool(name="w", bufs=1) as wp,
            tc.tile_pool(name="x", bufs=3) as xp,
            tc.tile_pool(name="o", bufs=3) as op,
            tc.tile_pool(name="ps", bufs=4, space="PSUM") as pp,
        ):
            wg = wp.tile([KP, C * MMB], F32)
            nc.sync.dma_start(out=wg[:], in_=wbd_g)
            wpt = wp.tile([KP, C * MMB], F32)
            nc.sync.dma_start(out=wpt[:], in_=wbd_p)
            for i in range(NL1):
                r0 = i * L1ROWS
                lhs = xp.tile([KP, 128], F32)
                nc.sync.dma_start(
                    out=lhs[:],
                    in_=_dram_v(xt, r0, [[128, MMB], [NPAD1, CIN3], [1, 128]]),
                )
                psg = pp.tile([128, C * MMB], F32, tag="psg")
                nc.tensor.matmul(psg[:], lhsT=lhs[:], rhs=wg[:], start=True, stop=True)
                psp = pp.tile([128, C * MMB], F32, tag="psp")
                nc.tensor.matmul(psp[:], lhsT=lhs[:], rhs=wpt[:], start=True, stop=True)
                og = op.tile([128, C * MMB], F32, tag="og")
                nc.scalar.copy(out=og[:], in_=psg[:])
                opt = op.tile([128, C * MMB], F32, tag="opt")
                nc.scalar.copy(out=opt[:], in_=psp[:])
                dst = [[C, 128], [128 * C, MMB], [1, C]]
                nc.sync.dma_start(out=_dram_v(g_out, r0 * C, dst), in_=og[:])
                nc.sync.dma_start(out=_dram_v(p_out, r0 * C, dst), in_=opt[:])
    nc.compile()
    return nc


# --------------------------------------------------------------------------- L2
def _build_l2():
    nc = bacc.Bacc("TRN2", target_bir_lowering=False, debug=False, num_devices=1)
    table = nc.dram_tensor("table", [N + 1, C], F32, kind="ExternalInput").ap()
    idx = nc.dram_tensor("idx", [NT, 128, SLOTS], I32, kind="ExternalInput").ap()
    p1t = nc.dram_tensor("p1t", [NT, 128, CP * C], F32, kind="ExternalInput").ap()
    dump = nc.dram_tensor("dump", [NT, 128, FR], F16, kind="ExternalOutput").ap()
    stats = nc.dram_tensor("stats", [128, 5 * C], F32, kind="ExternalOutput").ap()

    with tile.TileContext(nc) as tc:
        with (
            tc.tile_pool(name="g", bufs=2) as gp,
            tc.tile_pool(name="h", bufs=2) as hp,
            tc.tile_pool(name="i", bufs=2) as ip,
            tc.tile_pool(name="p1", bufs=2) as p1p,
            tc.tile_pool(name="sc", bufs=2) as scp,
            tc.tile_pool(name="sq1", bufs=1) as sqp,
            tc.tile_pool(name="acc", bufs=1) as accp,
        ):
            acc = accp.tile([128, 5 * C], F32)
            nc.vector.memset(acc[:], 0.0)
            for t in range(NT):
                it = ip.tile([128, SLOTS], I32)
                nc.sync.dma_start(out=it[:], in_=idx[t])
                p1 = p1p.tile([128, CP * C], F32)
                nc.sync.dma_start(out=p1[:], in_=p1t[t])
                r = gp.tile([128, FR], F32)
                _gather(nc, r, table, it)
                # fp16 copy of the gathered rows for the dump (ACT engine)
                r16 = hp.tile([128, FR], F16)
                nc.scalar.copy(out=r16[:], in_=r[:])
                # S[j,c] = sum_k r[j,k,c]
                sv = scp.tile([128, CP * C], F32, tag="sv")
                nc.vector.tensor_reduce(
                    out=sv[:], in_=_v(r[:], [[KNN * C, CP], [1, C], [C, KNN]]),
                    axis=AXX, op=ADD)
                # accA += sum_j S
                rA = scp.tile([128, C], F32, tag="rA")
                nc.vector.tensor_reduce(
                    out=rA[:], in_=_v(sv[:], [[1, C], [C, CP]]), axis=AXX, op=ADD)
                nc.vector.tensor_add(out=acc[:, 0:C], in0=acc[:, 0:C], in1=rA[:])
                # accB += sum_j P1*S
                ps = scp.tile([128, CP * C], F32, tag="ps")
                nc.vector.tensor_tensor(out=ps[:], in0=p1[:], in1=sv[:], op=MUL)
                rB = scp.tile([128, C], F32, tag="rB")
                nc.vector.tensor_reduce(
                    out=rB[:], in_=_v(ps[:], [[1, C], [C, CP]]), axis=AXX, op=ADD)
                nc.vector.tensor_add(out=acc[:, C:2 * C], in0=acc[:, C:2 * C], in1=rB[:])
                # accQ += sum_{j,k} r^2
                sq = sqp.tile([128, FR], F32, tag="sq")
                nc.scalar.square(out=sq[:], in_=r[:])
                rQ = scp.tile([128, C], F32, tag="rQ")
                nc.vector.tensor_reduce(
                    out=rQ[:], in_=_v(sq[:], [[1, C], [C, SLOTS]]), axis=AXX, op=ADD)
                nc.vector.tensor_add(out=acc[:, 2 * C:3 * C], in0=acc[:, 2 * C:3 * C], in1=rQ[:])
                # accP += sum_j P1 ; accPQ += sum_j P1^2
                rP = scp.tile([128, C], F32, tag="rP")
                nc.vector.tensor_reduce(
                    out=rP[:], in_=_v(p1[:], [[1, C], [C, CP]]), axis=AXX, op=ADD)
                nc.vector.tensor_add(out=acc[:, 3 * C:4 * C], in0=acc[:, 3 * C:4 * C], in1=rP[:])
                p1s = scp.tile([128, CP * C], F32, tag="p1s")
                nc.scalar.square(out=p1s[:], in_=p1[:])
                rPQ = scp.tile([128, C], F32, tag="rPQ")
                nc.vector.tensor_reduce(
                    out=rPQ[:], in_=_v(p1s[:], [[1, C], [C, CP]]), axis=AXX, op=ADD)
                nc.vector.tensor_add(out=acc[:, 4 * C:5 * C], in0=acc[:, 4 * C:5 * C], in1=rPQ[:])
                nc.sync.dma_start(out=dump[t], in_=r16[:])
            nc.sync.dma_start(out=stats, in_=acc[:])
    nc.compile()
    return nc


# --------------------------------------------------------------------------- L3
def _build_l3():
    nc = bacc.Bacc("TRN2", target_bir_lowering=False, debug=False, num_devices=1)
    dump = nc.dram_tensor("dump", [NT, 128, FR], F16, kind="ExternalInput").ap()
    p1t = nc.dram_tensor("p1t", [NT, 128, CP * C], F32, kind="ExternalInput").ap()
    a1r = nc.dram_tensor("a1r", [128, C], F32, kind="ExternalInput").ap()
    asqr = nc.dram_tensor("asqr", [128, C], F32, kind="ExternalInput").ap()
    dbr = nc.dram_tensor("dbr", [128, C], F32, kind="ExternalInput").ap()
    w_out = nc.dram_tensor("w_out", [NT, 128, SLOTS], F16, kind="ExternalOutput").ap()
    x2_out = nc.dram_tensor("x2_out", [NT, 128, CP * C], F16, kind="ExternalOutput").ap()

    with tile.TileContext(nc) as tc:
        with (
            tc.tile_pool(name="g", bufs=2) as gp,
            tc.tile_pool(name="big", bufs=2) as bigp,
            tc.tile_pool(name="p1", bufs=2) as p1p,
            tc.tile_pool(name="sc", bufs=2) as scp,
            tc.tile_pool(name="cst", bufs=1) as cst,
        ):
            a1 = cst.tile([128, C], F32)
            nc.sync.dma_start(out=a1[:], in_=a1r)
            asq = cst.tile([128, C], F32)
            nc.sync.dma_start(out=asq[:], in_=asqr)
            db = cst.tile([128, C], F32)
            nc.sync.dma_start(out=db[:], in_=dbr)

            # Two-stage software pipeline so the Pool w-reduce of tile t
            # overlaps the DVE tail (pw2/s/x2) of tile t-1.
            st = {}

            def stage_a(t):
                r16 = gp.tile([128, FR], F16, tag="r16")
                nc.sync.dma_start(out=r16[:], in_=dump[t])
                p1 = p1p.tile([128, CP * C], F32)
                nc.sync.dma_start(out=p1[:], in_=p1t[t])
                # q'[j,c] = P1 - d1/a1   (per point j, channel c)
                qp = scp.tile([128, CP * C], F16, tag="qp")
                nc.vector.tensor_tensor(
                    out=qp[:], in0=p1[:],
                    in1=_v(db[:], [[0, CP], [1, C]]), op=SUB)
                # u = r - q'
                u = bigp.tile([128, FR], F16, tag="u")
                nc.vector.tensor_tensor(
                    out=u[:], in0=r16[:],
                    in1=_v(qp[:], [[C, CP], [0, KNN], [1, C]]), op=SUB)
                # z[j,c] = a1^2 * u[j,0,c]
                z = scp.tile([128, CP * C], F16, tag="z")
                nc.vector.tensor_tensor(
                    out=z[:], in0=_v(u[:], [[KNN * C, CP], [1, C]]),
                    in1=_v(asq[:], [[0, CP], [1, C]]), op=MUL)
                # w[j,k] = sum_c u[j,k,c]*z[j,c]
                pw = bigp.tile([128, FR], F16, tag="pw")
                nc.vector.tensor_tensor(
                    out=pw[:], in0=u[:],
                    in1=_v(z[:], [[C, CP], [0, KNN], [1, C]]), op=MUL)
                w = scp.tile([128, SLOTS], F16, tag="w")
                with nc.allow_low_precision(reason="20-term fp16 dot, 2e-2 tol"):
                    nc.vector.tensor_reduce(
                        out=w[:], in_=_v(pw[:], [[KNN * C, CP], [C, KNN], [1, C]]),
                        axis=AXX, op=ADD)
                st[t] = (u, w)

            def stage_b(t):
                u, w = st.pop(t)
                # s[j,c] = sum_k u[j,k,c]*w[j,k] ; x2 = a1*s
                pw2 = bigp.tile([128, FR], F16, tag="pw2")
                nc.vector.tensor_tensor(
                    out=pw2[:], in0=u[:],
                    in1=_v(w[:], [[KNN, CP], [1, KNN], [0, C]]), op=MUL)
                s = scp.tile([128, CP * C], F16, tag="s")
                with nc.allow_low_precision(reason="9-term fp16 sum, 2e-2 tol"):
                    nc.vector.tensor_reduce(
                        out=s[:], in_=_v(pw2[:], [[KNN * C, CP], [1, C], [C, KNN]]),
                        axis=AXX, op=ADD)
                x2 = scp.tile([128, CP * C], F16, tag="x2")
                nc.vector.tensor_tensor(
                    out=x2[:], in0=s[:],
                    in1=_v(a1[:], [[0, CP], [1, C]]), op=MUL)
                nc.sync.dma_start(out=w_out[t], in_=w[:])
                nc.sync.dma_start(out=x2_out[t], in_=x2[:])

            stage_a(0)
            for t in range(1, NT):
                stage_a(t)
                stage_b(t - 1)
            stage_b(NT - 1)
    nc.compile()
    return nc


# --------------------------------------------------------------------------- L4
def _build_l4():
    nc = bacc.Bacc("TRN2", target_bir_lowering=False, debug=False, num_devices=1)
    table = nc.dram_tensor("table", [N + 1, C], F16, kind="ExternalInput").ap()
    idx = nc.dram_tensor("idx", [NT, 128, SLOTS], I32, kind="ExternalInput").ap()
    w_in = nc.dram_tensor("w_in", [NT, 128, SLOTS], F16, kind="ExternalInput").ap()
    x3_out = nc.dram_tensor("x3_out", [NT, 128, CP * C], F32, kind="ExternalOutput").ap()
    stats = nc.dram_tensor("stats", [128, 2 * C], F32, kind="ExternalOutput").ap()

    with tile.TileContext(nc) as tc:
        with (
            tc.tile_pool(name="g", bufs=2) as gp,
            tc.tile_pool(name="i", bufs=2) as ip,
            tc.tile_pool(name="w", bufs=2) as wp,
            tc.tile_pool(name="px", bufs=2) as pxp,
            tc.tile_pool(name="sc", bufs=2) as scp,
            tc.tile_pool(name="acc", bufs=1) as accp,
        ):
            acc = accp.tile([128, 2 * C], F32)
            nc.vector.memset(acc[:], 0.0)
            for t in range(NT):
                it = ip.tile([128, SLOTS], I32)
                nc.sync.dma_start(out=it[:], in_=idx[t])
                wt = wp.tile([128, SLOTS], F16)
                nc.sync.dma_start(out=wt[:], in_=w_in[t])
                r = gp.tile([128, FR], F16)
                _gather(nc, r, table, it)
                # px = r * w  (broadcast over c; f32 — the product tail can
                # exceed fp16 range)
                px = pxp.tile([128, FR], F32, tag="px")
                nc.vector.tensor_tensor(
                    out=px[:], in0=r[:],
                    in1=_v(wt[:], [[KNN, CP], [1, KNN], [0, C]]), op=MUL)
                x3 = scp.tile([128, CP * C], F32, tag="x3")
                nc.vector.tensor_reduce(
                    out=x3[:], in_=_v(px[:], [[KNN * C, CP], [1, C], [C, KNN]]),
                    axis=AXX, op=ADD)
                rA = scp.tile([128, C], F32, tag="rA")
                nc.vector.tensor_reduce(
                    out=rA[:], in_=_v(x3[:], [[1, C], [C, CP]]), axis=AXX, op=ADD)
                nc.vector.tensor_add(out=acc[:, 0:C], in0=acc[:, 0:C], in1=rA[:])
                sq = scp.tile([128, CP * C], F32, tag="sq")
                nc.scalar.square(out=sq[:], in_=x3[:])
                rB = scp.tile([128, C], F32, tag="rB")
                nc.vector.tensor_reduce(
                    out=rB[:], in_=_v(sq[:], [[1, C], [C, CP]]), axis=AXX, op=ADD)
                nc.vector.tensor_add(out=acc[:, C:2 * C], in0=acc[:, C:2 * C], in1=rB[:])
                nc.sync.dma_start(out=x3_out[t], in_=x3[:])
            nc.sync.dma_start(out=stats, in_=acc[:])
    nc.compile()
    return nc


# --------------------------------------------------------------------------- L5
def _build_l5():
    nc = bacc.Bacc("TRN2", target_bir_lowering=False, debug=False, num_devices=1)
    x3t = nc.dram_tensor("x3t", [C, NPC], F32, kind="ExternalInput")
    ft = nc.dram_tensor("ft", [C, NPC], F32, kind="ExternalInput")
    wr1t = nc.dram_tensor("wr1t", [2 * C, C], F32, kind="ExternalInput").ap()
    a2 = nc.dram_tensor("a2", [C, 1], F32, kind="ExternalInput").ap()
    d2 = nc.dram_tensor("d2", [C, 1], F32, kind="ExternalInput").ap()
    br1 = nc.dram_tensor("br1", [C, 1], F32, kind="ExternalInput").ap()
    tt_out = nc.dram_tensor("tt_out", [C, NPC], F32, kind="ExternalOutput")
    stats = nc.dram_tensor("stats", [C, 2], F32, kind="ExternalOutput").ap()

    nq = -(-NPC // QCH)
    with tile.TileContext(nc) as tc:
        with (
            tc.tile_pool(name="u", bufs=3) as up,
            tc.tile_pool(name="o", bufs=3) as op,
            tc.tile_pool(name="sc", bufs=3) as scp,
            tc.tile_pool(name="cst", bufs=1) as cst,
            tc.tile_pool(name="acc", bufs=1) as accp,
            tc.tile_pool(name="ps", bufs=4, space="PSUM") as pp,
        ):
            w1 = cst.tile([2 * C, C], F32)
            nc.sync.dma_start(out=w1[:], in_=wr1t)
            ca2 = cst.tile([C, 1], F32)
            nc.sync.dma_start(out=ca2[:], in_=a2)
            cd2 = cst.tile([C, 1], F32)
            nc.sync.dma_start(out=cd2[:], in_=d2)
            cbr = cst.tile([C, 1], F32)
            nc.sync.dma_start(out=cbr[:], in_=br1)
            acc = accp.tile([C, 2], F32)
            nc.vector.memset(acc[:], 0.0)
            for i in range(nq):
                c0 = i * QCH
                qn = min(QCH, NPC - c0)
                u = up.tile([2 * C, QCH], F32)
                nc.sync.dma_start(
                    out=u[0:C, :qn], in_=_dram_v(x3t, c0, [[NPC, C], [1, qn]]))
                nc.sync.dma_start(
                    out=u[C:2 * C, :qn], in_=_dram_v(ft, c0, [[NPC, C], [1, qn]]))
                nc.scalar.activation(
                    out=u[0:C, :qn], in_=u[0:C, :qn],
                    func=mybir.ActivationFunctionType.Relu,
                    bias=cd2[:], scale=ca2[:])
                ps = pp.tile([C, QCH], F32)
                nc.tensor.matmul(ps[:, :qn], lhsT=w1[:], rhs=u[:, :qn],
                                 start=True, stop=True)
                tt = op.tile([C, QCH], F32)
                nc.vector.tensor_scalar_add(out=tt[:, :qn], in0=ps[:, :qn], scalar1=cbr[:])
                rs = scp.tile([C, 1], F32, tag="rs")
                nc.vector.tensor_reduce(out=rs[:], in_=tt[:, :qn], axis=AXX, op=ADD)
                nc.vector.tensor_add(out=acc[:, 0:1], in0=acc[:, 0:1], in1=rs[:])
                sq = scp.tile([C, QCH], F32, tag="sq")
                nc.scalar.square(out=sq[:, :qn], in_=tt[:, :qn])
                rq = scp.tile([C, 1], F32, tag="rq")
                nc.vector.tensor_reduce(out=rq[:], in_=sq[:, :qn], axis=AXX, op=ADD)
                nc.vector.tensor_add(out=acc[:, 1:2], in0=acc[:, 1:2], in1=rq[:])
                nc.sync.dma_start(
                    out=_dram_v(tt_out, c0, [[NPC, C], [1, qn]]), in_=tt[:, :qn])
            nc.sync.dma_start(out=stats, in_=acc[:])
    nc.compile()
    return nc


# --------------------------------------------------------------------------- L6
def _build_l6():
    nc = bacc.Bacc("TRN2", target_bir_lowering=False, debug=False, num_devices=1)
    ttin = nc.dram_tensor("ttin", [C, NPC], F32, kind="ExternalInput")
    wr2t = nc.dram_tensor("wr2t", [C, C], F32, kind="ExternalInput").ap()
    a3 = nc.dram_tensor("a3", [C, 1], F32, kind="ExternalInput").ap()
    d3 = nc.dram_tensor("d3", [C, 1], F32, kind="ExternalInput").ap()
    br2 = nc.dram_tensor("br2", [C, 1], F32, kind="ExternalInput").ap()
    outt = nc.dram_tensor("outt", [C, NPC], F32, kind="ExternalOutput")

    nq = -(-NPC // QCH)
    with tile.TileContext(nc) as tc:
        with (
            tc.tile_pool(name="u", bufs=3) as up,
            tc.tile_pool(name="o", bufs=3) as op,
            tc.tile_pool(name="cst", bufs=1) as cst,
            tc.tile_pool(name="ps", bufs=4, space="PSUM") as pp,
        ):
            w2 = cst.tile([C, C], F32)
            nc.sync.dma_start(out=w2[:], in_=wr2t)
            ca3 = cst.tile([C, 1], F32)
            nc.sync.dma_start(out=ca3[:], in_=a3)
            cd3 = cst.tile([C, 1], F32)
            nc.sync.dma_start(out=cd3[:], in_=d3)
            cbr = cst.tile([C, 1], F32)
            nc.sync.dma_start(out=cbr[:], in_=br2)
            for i in range(nq):
                c0 = i * QCH
                qn = min(QCH, NPC - c0)
                u = up.tile([C, QCH], F32)
                nc.sync.dma_start(
                    out=u[:, :qn], in_=_dram_v(ttin, c0, [[NPC, C], [1, qn]]))
                nc.scalar.activation(
                    out=u[:, :qn], in_=u[:, :qn],
                    func=mybir.ActivationFunctionType.Relu,
                    bias=cd3[:], scale=ca3[:])
                ps = pp.tile([C, QCH], F32)
                nc.tensor.matmul(ps[:, :qn], lhsT=w2[:], rhs=u[:, :qn],
                                 start=True, stop=True)
                ot = op.tile([C, QCH], F32)
                nc.vector.tensor_scalar_add(out=ot[:, :qn], in0=ps[:, :qn], scalar1=cbr[:])
                nc.sync.dma_start(
                    out=_dram_v(outt, c0, [[NPC, C], [1, qn]]), in_=ot[:, :qn])
    nc.compile()
    return nc


def _prog(name):
    if name not in _prog_cache:
        _prog_cache[name] = {
            "l1": _build_l1, "l2": _build_l2, "l3": _build_l3,
            "l4": _build_l4, "l5": _build_l5, "l6": _build_l6,
        }[name]()
    return _prog_cache[name]


def _run(name, in_maps):
    nc = _prog(name)
    res = run_bass_kernel_spmd(nc, in_maps, core_ids=list(range(NCORES)),
                               trace=PROFILE)
    if PROFILE:
        _last_exec_ns[name] = res.exec_time_ns
    return res.results


# ------------------------------------------------------------------------ host
def kernel(points, feature, index, W1, g1, b1, g2, b2, Wr1, br1, g3, b3, Wr2, br2):
    points = np.asarray(points, np.float32)
    feature = np.asarray(feature, np.float32)
    index = np.asarray(index)
    f32 = np.float32

    # ---- L1: G = X @ W1cat.T and P1 = points @ W1x.T, per-core rows ----
    w1cat_t = np.ascontiguousarray(np.asarray(W1, f32).T)          # [23, 20]
    w1x_t = np.zeros((CIN3, C), f32)
    w1x_t[C:, :] = w1cat_t[C:, :]
    wbd_g = np.zeros((CIN3 * MMB, C * MMB), f32)
    wbd_p = np.zeros((CIN3 * MMB, C * MMB), f32)
    for b in range(MMB):
        wbd_g[CIN3 * b:CIN3 * (b + 1), C * b:C * (b + 1)] = w1cat_t
        wbd_p[CIN3 * b:CIN3 * (b + 1), C * b:C * (b + 1)] = w1x_t

    in_maps = []
    for c in range(NCORES):
        sl = slice(c * NPC, (c + 1) * NPC)
        xt = np.zeros((CIN3, NPAD1), f32)
        xt[:C, :NPC] = feature[sl].T
        xt[C:, :NPC] = points[sl].T
        in_maps.append({"xt": xt, "wbd_g": wbd_g, "wbd_p": wbd_p})
    r1 = _run("l1", in_maps)

    g_full = np.zeros((N + 1, C), f32)
    p1 = np.zeros((NCORES, NPAD, C), f32)
    for c in range(NCORES):
        g_full[c * NPC:(c + 1) * NPC] = r1[c]["g_out"][:NPC]
        p1[c, :NPC] = r1[c]["p_out"][:NPC]
    p1t = p1.reshape(NCORES, NT, 128, CP * C)

    # ---- gather-layout index prep (int32, padded with zero-row index N) ----
    idxs = []
    for c in range(NCORES):
        ip = np.full((NPAD, KNN), N, np.int32)
        ip[:NPC] = index[c * NPC:(c + 1) * NPC].astype(np.int32)
        idxs.append(ip.reshape(NT, 128, SLOTS))

    # ---- L2: gather G, bn1 partial stats, dump fp16 ----
    in_maps = [{"table": g_full, "idx": idxs[c], "p1t": p1t[c]} for c in range(NCORES)]
    r2 = _run("l2", in_maps)

    sA = np.zeros(C, np.float64); sB = np.zeros(C, np.float64)
    sQ = np.zeros(C, np.float64); sP = np.zeros(C, np.float64)
    sPQ = np.zeros(C, np.float64)
    for c in range(NCORES):
        st = r2[c]["stats"].astype(np.float64)
        sA += st[:, 0:C].sum(0); sB += st[:, C:2 * C].sum(0)
        sQ += st[:, 2 * C:3 * C].sum(0); sP += st[:, 3 * C:4 * C].sum(0)
        sPQ += st[:, 4 * C:5 * C].sum(0)
    cnt = float(N * KNN)
    sx = sA - KNN * sP
    sxx = sQ - 2.0 * sB + KNN * sPQ
    m1 = sx / cnt
    v1 = sxx / cnt - m1 * m1
    a1 = np.asarray(g1, np.float64) / np.sqrt(v1 + EPS)
    d1 = np.asarray(b1, np.float64) - m1 * a1

    a1r = np.broadcast_to(a1.astype(f32), (128, C)).copy()
    asqr = np.broadcast_to((a1 * a1).astype(f32), (128, C)).copy()
    dbr = np.broadcast_to((d1 / a1).astype(f32), (128, C)).copy()

    # ---- L3: attention weights + first aggregation ----
    in_maps = [{"dump": r2[c]["dump"], "p1t": p1t[c], "a1r": a1r,
                "asqr": asqr, "dbr": dbr}
               for c in range(NCORES)]
    r3 = _run("l3", in_maps)

    x2_full = np.zeros((N + 1, C), np.float16)
    for c in range(NCORES):
        x2_full[c * NPC:(c + 1) * NPC] = \
            np.asarray(r3[c]["x2_out"]).reshape(NPAD, C)[:NPC]

    # ---- L4: gather x2, second aggregation, bn2 partial stats ----
    in_maps = [{"table": x2_full, "idx": idxs[c], "w_in": r3[c]["w_out"]}
               for c in range(NCORES)]
    r4 = _run("l4", in_maps)

    s3 = np.zeros(C, np.float64); s3q = np.zeros(C, np.float64)
    x3t_cores = []
    for c in range(NCORES):
        st = r4[c]["stats"].astype(np.float64)
        s3 += st[:, 0:C].sum(0); s3q += st[:, C:2 * C].sum(0)
        x3t_cores.append(np.ascontiguousarray(
            r4[c]["x3_out"].reshape(NPAD, C)[:NPC].T))
    m2 = s3 / N
    v2 = s3q / N - m2 * m2
    a2 = np.asarray(g2, np.float64) / np.sqrt(v2 + EPS)
    d2 = np.asarray(b2, np.float64) - m2 * a2

    # ---- L5: t = relu(bn2(x3)) ++ feature @ Wr1.T + br1, bn3 partial stats ----
    wr1t = np.ascontiguousarray(np.asarray(Wr1, f32).T)            # [40, 20]
    in_maps = []
    for c in range(NCORES):
        in_maps.append({
            "x3t": x3t_cores[c],
            "ft": np.ascontiguousarray(feature[c * NPC:(c + 1) * NPC].T),
            "wr1t": wr1t,
            "a2": a2.astype(f32).reshape(C, 1),
            "d2": d2.astype(f32).reshape(C, 1),
            "br1": np.asarray(br1, f32).reshape(C, 1),
        })
    r5 = _run("l5", in_maps)

    sT = np.zeros(C, np.float64); sTq = np.zeros(C, np.float64)
    for c in range(NCORES):
        st = r5[c]["stats"].astype(np.float64)
        sT += st[:, 0]; sTq += st[:, 1]
    m3 = sT / N
    v3 = sTq / N - m3 * m3
    a3 = np.asarray(g3, np.float64) / np.sqrt(v3 + EPS)
    d3 = np.asarray(b3, np.float64) - m3 * a3

    # ---- L6: out = relu(bn3(t)) @ Wr2.T + br2 ----
    wr2t = np.ascontiguousarray(np.asarray(Wr2, f32).T)            # [20, 20]
    in_maps = []
    for c in range(NCORES):
        in_maps.append({
            "ttin": r5[c]["tt_out"],
            "wr2t": wr2t,
            "a3": a3.astype(f32).reshape(C, 1),
            "d3": d3.astype(f32).reshape(C, 1),
            "br2": np.asarray(br2, f32).reshape(C, 1),
        })
    r6 = _run("l6", in_maps)

    out = np.empty((N, C), f32)
    for c in range(NCORES):
        out[c * NPC:(c + 1) * NPC] = r6[c]["outt"].T
    return out
